# revision 8
# baseline (speedup 1.0000x reference)
"""HQQ 4-bit quantized linear on 8 Trainium2 NeuronCores (Bass/Tile).

out[4096, 11008] = x[4096, 4096] @ dequant(W_q, scale, zero).T + bias

Index fact: reference reshapes ((W_r - zero) * scale) from [64, 704512] to
[11008, 4096].  With o = output feature, i = input feature:
    o = g_row * 172 + j,   group g = j * 4096 + i,   g_row in [0, 64)
g_rows 0..31 come from the HIGH nibble of W_q rows 0..31, g_rows 32..63 from
the LOW nibble of the same rows.  Core c takes W_q rows [4c, 4c+4) and
extracts BOTH nibbles -> output cols [688c, 688c+688) (hi) and
[5504+688c, 5504+688c+688) (lo).  Each W_q byte is read exactly once.

Per-core pipeline (PE does nothing but matmuls):
  host: stage W_q transposed [4096(i), 4(r)*172(j)] and scale/zero
        transposed [4096(i), 172(j)] so dequant writes W.T directly.
  phase 1: per 128-row i-block k: DMA q/scale/zero, dequant on DVE (hi
        nibble) + Pool (lo nibble) into resident WT[128, 32, 1376] fp16:
          hi: (q & 240) * (s/16) - z*s      lo: (q & 15) * s - z*s
  phase 2: per 128-token tile: DMA x, fp32->fp16 on Act, xbar DMA-transpose
        to x.T tiles, accumulate out = sum_k xT[k].T @ WT[k] in PSUM (bias
        preloaded via K=1 ones x bias matmul), DMA PSUM -> DRAM directly.
"""

import numpy as np
from contextlib import ExitStack

import concourse.bacc as bacc
import concourse.bass as bass
import concourse.mybir as mybir
import concourse.tile as tile
from concourse.bass_utils import run_bass_kernel_spmd

dt = mybir.dt
Alu = mybir.AluOpType

TOKENS, IN_F, OUT_F, GS = 4096, 4096, 11008, 64
G = OUT_F * IN_F // GS            # 704512 quantization groups
J = G // IN_F                     # 172 groups per (g_row, i) plane
NCORES = 8
RPC = 4                           # W_q rows per core (both nibbles)
O_HALF = RPC * J                  # 688 output cols per nibble block
O_C = 2 * O_HALF                  # 1376 output cols per core
NT = TOKENS // 128                # 32 token tiles
NK = IN_F // 128                  # 32 contraction blocks
O_SPLITS = ((0, 512), (512, 512), (1024, 352))   # psum o-tiles (1 bank each)
XC = 2048                         # x i-chunk (half a row-block)

_CACHE = {}


def _build():
    nc = bacc.Bacc("TRN2", target_bir_lowering=False, debug=False,
                   num_devices=NCORES)

    x_d = nc.dram_tensor("x", [TOKENS, IN_F], dt.float32, kind="ExternalInput")
    q_d = nc.dram_tensor("wqt", [IN_F, O_HALF], dt.int32, kind="ExternalInput")
    s_d = nc.dram_tensor("st", [IN_F, O_HALF], dt.float32, kind="ExternalInput")
    z_d = nc.dram_tensor("zt", [IN_F, O_HALF], dt.float32, kind="ExternalInput")
    b_d = nc.dram_tensor("bias", [1, O_C], dt.float32, kind="ExternalInput")
    o_d = nc.dram_tensor("out", [TOKENS, O_C], dt.float32, kind="ExternalOutput")

    with ExitStack() as ctx:
        tc = ctx.enter_context(tile.TileContext(nc))
        const = ctx.enter_context(tc.tile_pool(name="const", bufs=1))
        p1 = ctx.enter_context(tc.tile_pool(name="p1", bufs=2))
        px = ctx.enter_context(tc.tile_pool(name="px", bufs=2))
        pxt = ctx.enter_context(tc.tile_pool(name="pxt", bufs=4))
        po = ctx.enter_context(tc.tile_pool(name="po", bufs=2))
        pacc = ctx.enter_context(
            tc.tile_pool(name="pacc", bufs=2, space=bass.MemorySpace.PSUM))

        biasf = const.tile([1, O_C], dt.float32)
        nc.sync.dma_start(biasf[:], b_d[:])
        biash = const.tile([1, O_C], dt.float16)
        nc.scalar.copy(biash[:], biasf[:])
        ones = const.tile([1, 128], dt.float16)
        nc.vector.memset(ones[:], 1.0)

        # resident transposed dequantized weights: [i-partition, k-block, o]
        WT = const.tile([128, NK, O_C], dt.float16)

        # ---- phase 1: dequant W.T into resident fp16 WT ----
        # hi = q>>4, lo = q&15;  w = nib*s - z*s
        #   lo_i = q & 15                  (DVE, int32)
        #   t_lo = lo_i * s                (DVE)
        #   u    = (q * 1/16) * s          (DVE fused)
        #   v    = u - t_lo/16  = hi * s   (DVE fused)
        #   zs   = z * s                   (Pool)
        #   WT_hi = v - zs, WT_lo = t_lo - zs   (Pool, fp16 out)
        for k in range(NK):
            i0 = k * 128
            st = p1.tile([128, O_HALF], dt.float32, tag="s")
            nc.sync.dma_start(st[:], s_d[i0:i0 + 128, :])
            zt = p1.tile([128, O_HALF], dt.float32, tag="z")
            nc.sync.dma_start(zt[:], z_d[i0:i0 + 128, :])
            q = p1.tile([128, O_HALF], dt.int32, tag="q")
            nc.sync.dma_start(q[:], q_d[i0:i0 + 128, :])
            zs = p1.tile([128, O_HALF], dt.float32, tag="zs")
            nc.gpsimd.tensor_mul(zs[:], zt[:], st[:])
            lo_i = p1.tile([128, O_HALF], dt.int32, tag="loi")
            nc.vector.tensor_single_scalar(lo_i[:], q[:], 15, Alu.bitwise_and)
            t_lo = p1.tile([128, O_HALF], dt.float32, tag="tlo")
            nc.vector.tensor_mul(t_lo[:], lo_i[:], st[:])
            u = p1.tile([128, O_HALF], dt.float32, tag="u")
            nc.vector.scalar_tensor_tensor(
                u[:], q[:], 1.0 / 16.0, st[:], Alu.mult, Alu.mult)
            v = p1.tile([128, O_HALF], dt.float32, tag="v")
            nc.vector.scalar_tensor_tensor(
                v[:], t_lo[:], -1.0 / 16.0, u[:], Alu.mult, Alu.add)
            nc.gpsimd.tensor_sub(WT[:, k, 0:O_HALF], v[:], zs[:])
            nc.gpsimd.tensor_sub(WT[:, k, O_HALF:O_C], t_lo[:], zs[:])

        # ---- phase 2: stream x, xbar-transpose, matmul, psum->dram ----
        for t in range(NT):
            acc = []
            for p, (ob, on) in enumerate(O_SPLITS):
                a = pacc.tile([128, on], dt.float32, tag=f"a{p}")
                nc.tensor.matmul(
                    a[:], ones[0:1, :], biash[0:1, ob:ob + on],
                    start=True, stop=False)
                acc.append(a)
            for h in range(IN_F // XC):
                xr = px.tile([128, XC], dt.float32, tag="xr")
                nc.scalar.dma_start(
                    xr[:], x_d[t * 128:(t + 1) * 128, h * XC:(h + 1) * XC])
                xh = px.tile([128, XC], dt.float16, tag="xh")
                nc.scalar.copy(xh[:], xr[:])
                xT = pxt.tile([128, XC // 128, 128], dt.float16, tag="xT")
                nc.scalar.dma_start_transpose(xT[:], xh[:])
                for di in range(XC // 128):
                    k = h * (XC // 128) + di
                    for p, (ob, on) in enumerate(O_SPLITS):
                        nc.tensor.matmul(
                            acc[p][:], xT[:, di, :], WT[:, k, ob:ob + on],
                            start=False, stop=(k == NK - 1))
            for p, (ob, on) in enumerate(O_SPLITS):
                ot = po.tile([128, on], dt.float32, tag=f"o{p}")
                nc.vector.tensor_copy(ot[:], acc[p][:])
                nc.sync.dma_start(
                    o_d[t * 128:(t + 1) * 128, ob:ob + on], ot[:])

    nc.compile()
    return nc


def get_nc():
    if "nc" not in _CACHE:
        _CACHE["nc"] = _build()
    return _CACHE["nc"]


def make_in_maps(x, W_q, scale, zero, bias):
    x = np.ascontiguousarray(x, dtype=np.float32)
    W_q3 = np.asarray(W_q, dtype=np.int32).reshape(GS // 2, J, IN_F)
    s_t = np.ascontiguousarray(np.tile(
        np.asarray(scale, dtype=np.float32).reshape(J, IN_F).T, (1, RPC)))
    z_t = np.ascontiguousarray(np.tile(
        np.asarray(zero, dtype=np.float32).reshape(J, IN_F).T, (1, RPC)))
    bias = np.asarray(bias, dtype=np.float32)
    in_maps = []
    for c in range(NCORES):
        wqt = np.ascontiguousarray(
            W_q3[RPC * c:RPC * (c + 1)].transpose(2, 0, 1).reshape(IN_F, O_HALF))
        b2 = np.concatenate([
            bias[O_HALF * c:O_HALF * (c + 1)],
            bias[OUT_F // 2 + O_HALF * c:OUT_F // 2 + O_HALF * (c + 1)],
        ]).reshape(1, O_C)
        in_maps.append({
            "x": x, "wqt": wqt, "st": s_t, "zt": z_t, "bias": b2,
        })
    return in_maps


def assemble_out(results):
    out = np.empty((TOKENS, OUT_F), dtype=np.float32)
    for c in range(NCORES):
        r = results[c]["out"]
        out[:, O_HALF * c:O_HALF * (c + 1)] = r[:, :O_HALF]
        out[:, OUT_F // 2 + O_HALF * c:OUT_F // 2 + O_HALF * (c + 1)] = \
            r[:, O_HALF:]
    return out


def kernel(x, W_q, scale, zero, bias):
    nc = get_nc()
    in_maps = make_in_maps(x, W_q, scale, zero, bias)
    res = run_bass_kernel_spmd(nc, in_maps, list(range(NCORES)))
    return assemble_out(res.results)


# revision 10
# speedup vs baseline: 1.1119x; 1.1119x over previous
"""HQQ 4-bit quantized linear on 8 Trainium2 NeuronCores (Bass/Tile).

out[4096, 11008] = x[4096, 4096] @ dequant(W_q, scale, zero).T + bias

Index fact: reference reshapes ((W_r - zero) * scale) from [64, 704512] to
[11008, 4096].  With o = output feature, i = input feature:
    o = g_row * 172 + j,   group g = j * 4096 + i,   g_row in [0, 64)
g_rows 0..31 come from the HIGH nibble of W_q rows 0..31, g_rows 32..63 from
the LOW nibble of the same rows.  Core c takes W_q rows [4c, 4c+4) and
extracts BOTH nibbles -> output cols [688c, 688c+688) (hi) and
[5504+688c, 5504+688c+688) (lo).  Each W_q byte is read exactly once.

Host staging (pure layout/dtype-preserving transforms):
  wqt  uint8 [4096(i), 4(r)*172(j)]   (W_q values are bytes; transposed)
  st/zt fp32 [4096(i), 688]           (scale/zero transposed, tiled x4 over r)
  bias fp32 [1, 1376] = [hi block 688 | lo block 688]

Per-core pipeline (PE does nothing but matmuls):
  phase 1 (per 128-row i-block k): DMA q/s/z on SP queue; Act converts s,z
      to fp16; DVE extracts nibbles (1-byte shr/and), then fused
      (nib - z) * s in fp16 into resident WT[128, 32, 1376] fp16.
  phase 2 (per 128-token tile, x-prep prefetched 2 tiles ahead on Act
      queue): DMA x, fp32->fp16 on Act, xbar DMA-transpose to x.T tiles,
      PSUM-accumulate out = bias + sum_k xT[k].T @ WT[k] (bias preloaded
      via K=1 ones x bias matmul), DVE copy PSUM->SBUF, store on SP queue.
"""

import numpy as np
from contextlib import ExitStack

import concourse.bacc as bacc
import concourse.bass as bass
import concourse.mybir as mybir
import concourse.tile as tile
from concourse.bass_utils import run_bass_kernel_spmd

dt = mybir.dt
Alu = mybir.AluOpType

TOKENS, IN_F, OUT_F, GS = 4096, 4096, 11008, 64
G = OUT_F * IN_F // GS            # 704512 quantization groups
J = G // IN_F                     # 172 groups per (g_row, i) plane
NCORES = 8
RPC = 4                           # W_q rows per core (both nibbles)
O_HALF = RPC * J                  # 688 output cols per nibble block
O_C = 2 * O_HALF                  # 1376 output cols per core
NT = TOKENS // 128                # 32 token tiles
NK = IN_F // 128                  # 32 contraction blocks
O_SPLITS = ((0, 512), (512, 512), (1024, 352))   # psum o-tiles (1 bank each)
XC = 2048                         # x i-chunk (half a row-block)
NH = IN_F // XC                   # chunks per row-block
LOOKAHEAD = 2                     # x-prep prefetch distance (t-tiles)

_CACHE = {}


def _build():
    nc = bacc.Bacc("TRN2", target_bir_lowering=False, debug=False,
                   num_devices=NCORES)

    x_d = nc.dram_tensor("x", [TOKENS, IN_F], dt.float32, kind="ExternalInput")
    q_d = nc.dram_tensor("wqt", [IN_F, O_HALF], dt.uint8, kind="ExternalInput")
    s_d = nc.dram_tensor("st", [IN_F, O_HALF], dt.float32, kind="ExternalInput")
    z_d = nc.dram_tensor("zt", [IN_F, O_HALF], dt.float32, kind="ExternalInput")
    b_d = nc.dram_tensor("bias", [1, O_C], dt.float32, kind="ExternalInput")
    o_d = nc.dram_tensor("out", [TOKENS, O_C], dt.float32, kind="ExternalOutput")

    with ExitStack() as ctx:
        tc = ctx.enter_context(tile.TileContext(nc))
        const = ctx.enter_context(tc.tile_pool(name="const", bufs=1))
        p1 = ctx.enter_context(tc.tile_pool(name="p1", bufs=2))
        px = ctx.enter_context(tc.tile_pool(name="px", bufs=3))
        pxt = ctx.enter_context(tc.tile_pool(name="pxt", bufs=6))
        po = ctx.enter_context(tc.tile_pool(name="po", bufs=2))
        pacc = ctx.enter_context(
            tc.tile_pool(name="pacc", bufs=2, space=bass.MemorySpace.PSUM))

        biasf = const.tile([1, O_C], dt.float32)
        nc.scalar.dma_start(biasf[:], b_d[:])
        biash = const.tile([1, O_C], dt.float16)
        nc.scalar.copy(biash[:], biasf[:])
        ones = const.tile([1, 128], dt.float16)
        nc.vector.memset(ones[:], 1.0)

        # resident transposed dequantized weights: [i-partition, k-block, o]
        WT = const.tile([128, NK, O_C], dt.float16)

        def prefetch(t):
            """x row-block t: load (Act q), fp16 convert (Act), xbar
            transpose (Act q) into a [128(i%128), 16(i//128), 128(tok)]
            x.T tile per chunk."""
            xts = []
            for h in range(NH):
                xr = px.tile([128, XC], dt.float32, tag="xr")
                nc.scalar.dma_start(
                    xr[:], x_d[t * 128:(t + 1) * 128, h * XC:(h + 1) * XC])
                xh = px.tile([128, XC], dt.float16, tag="xh")
                nc.scalar.copy(xh[:], xr[:])
                xT = pxt.tile([128, XC // 128, 128], dt.float16, tag="xT")
                nc.scalar.dma_start_transpose(xT[:], xh[:])
                xts.append(xT)
            return xts

        # ---- phase 1: dequant W.T into resident fp16 WT (DVE only) ----
        #   hi = q >> 4, lo = q & 15;  w = (nib - z) * s
        for k in range(NK):
            i0 = k * 128
            st = p1.tile([128, O_HALF], dt.float32, tag="s")
            nc.sync.dma_start(st[:], s_d[i0:i0 + 128, :])
            zt = p1.tile([128, O_HALF], dt.float32, tag="z")
            nc.sync.dma_start(zt[:], z_d[i0:i0 + 128, :])
            q = p1.tile([128, O_HALF], dt.uint8, tag="q")
            nc.sync.dma_start(q[:], q_d[i0:i0 + 128, :])
            sh = p1.tile([128, O_HALF], dt.float16, tag="sh")
            nc.scalar.copy(sh[:], st[:])
            zh = p1.tile([128, O_HALF], dt.float16, tag="zh")
            nc.scalar.copy(zh[:], zt[:])
            hi_u = p1.tile([128, O_HALF], dt.uint8, tag="hi")
            nc.vector.tensor_single_scalar(
                hi_u[:], q[:], 4, Alu.logical_shift_right)
            lo_u = p1.tile([128, O_HALF], dt.uint8, tag="lo")
            nc.vector.tensor_single_scalar(lo_u[:], q[:], 15, Alu.bitwise_and)
            dhi = p1.tile([128, O_HALF], dt.float16, tag="dhi")
            nc.vector.scalar_tensor_tensor(
                dhi[:], hi_u[:], 1.0, zh[:], Alu.mult, Alu.subtract)
            dlo = p1.tile([128, O_HALF], dt.float16, tag="dlo")
            nc.vector.scalar_tensor_tensor(
                dlo[:], lo_u[:], 1.0, zh[:], Alu.mult, Alu.subtract)
            nc.vector.tensor_mul(WT[:, k, 0:O_HALF], dhi[:], sh[:])
            nc.vector.tensor_mul(WT[:, k, O_HALF:O_C], dlo[:], sh[:])

        # ---- phase 2: matmul over prefetched x.T tiles, psum->out ----
        inflight = [prefetch(t) for t in range(LOOKAHEAD)]
        for t in range(NT):
            if t + LOOKAHEAD < NT:
                inflight.append(prefetch(t + LOOKAHEAD))
            xts = inflight.pop(0)
            acc = []
            for p, (ob, on) in enumerate(O_SPLITS):
                a = pacc.tile([128, on], dt.float32, tag=f"a{p}")
                nc.tensor.matmul(
                    a[:], ones[0:1, :], biash[0:1, ob:ob + on],
                    start=True, stop=False)
                acc.append(a)
            for h in range(NH):
                for di in range(XC // 128):
                    k = h * (XC // 128) + di
                    for p, (ob, on) in enumerate(O_SPLITS):
                        nc.tensor.matmul(
                            acc[p][:], xts[h][:, di, :], WT[:, k, ob:ob + on],
                            start=False, stop=(k == NK - 1))
            for p, (ob, on) in enumerate(O_SPLITS):
                ot = po.tile([128, on], dt.float32, tag=f"o{p}")
                nc.vector.tensor_copy(ot[:], acc[p][:])
                nc.sync.dma_start(
                    o_d[t * 128:(t + 1) * 128, ob:ob + on], ot[:])

    nc.compile()
    return nc


def get_nc():
    if "nc" not in _CACHE:
        _CACHE["nc"] = _build()
    return _CACHE["nc"]


def make_in_maps(x, W_q, scale, zero, bias):
    x = np.ascontiguousarray(x, dtype=np.float32)
    W_q3 = np.asarray(W_q).astype(np.uint8).reshape(GS // 2, J, IN_F)
    s_t = np.ascontiguousarray(np.tile(
        np.asarray(scale, dtype=np.float32).reshape(J, IN_F).T, (1, RPC)))
    z_t = np.ascontiguousarray(np.tile(
        np.asarray(zero, dtype=np.float32).reshape(J, IN_F).T, (1, RPC)))
    bias = np.asarray(bias, dtype=np.float32)
    in_maps = []
    for c in range(NCORES):
        wqt = np.ascontiguousarray(
            W_q3[RPC * c:RPC * (c + 1)].transpose(2, 0, 1).reshape(IN_F, O_HALF))
        b2 = np.concatenate([
            bias[O_HALF * c:O_HALF * (c + 1)],
            bias[OUT_F // 2 + O_HALF * c:OUT_F // 2 + O_HALF * (c + 1)],
        ]).reshape(1, O_C)
        in_maps.append({
            "x": x, "wqt": wqt, "st": s_t, "zt": z_t, "bias": b2,
        })
    return in_maps


def assemble_out(results):
    out = np.empty((TOKENS, OUT_F), dtype=np.float32)
    for c in range(NCORES):
        r = results[c]["out"]
        out[:, O_HALF * c:O_HALF * (c + 1)] = r[:, :O_HALF]
        out[:, OUT_F // 2 + O_HALF * c:OUT_F // 2 + O_HALF * (c + 1)] = \
            r[:, O_HALF:]
    return out


def kernel(x, W_q, scale, zero, bias):
    nc = get_nc()
    in_maps = make_in_maps(x, W_q, scale, zero, bias)
    res = run_bass_kernel_spmd(nc, in_maps, list(range(NCORES)))
    return assemble_out(res.results)


# revision 12
# speedup vs baseline: 1.1262x; 1.0129x over previous
"""HQQ 4-bit quantized linear on 8 Trainium2 NeuronCores (Bass/Tile).

out[4096, 11008] = x[4096, 4096] @ dequant(W_q, scale, zero).T + bias

Index fact: reference reshapes ((W_r - zero) * scale) from [64, 704512] to
[11008, 4096].  With o = output feature, i = input feature:
    o = g_row * 172 + j,   group g = j * 4096 + i,   g_row in [0, 64)
g_rows 0..31 come from the HIGH nibble of W_q rows 0..31, g_rows 32..63 from
the LOW nibble of the same rows.  Core c takes W_q rows [4c, 4c+4) and
extracts BOTH nibbles -> output cols [688c, 688c+688) (hi) and
[5504+688c, 5504+688c+688) (lo).  Each W_q byte is read exactly once.

Host staging (pure layout/dtype-preserving transforms):
  wqt  uint8 [4096(i), 4(r)*172(j)]   (W_q values are bytes; transposed)
  st/zt fp32 [4096(i), 688]           (scale/zero transposed, tiled x4 over r)
  bias fp32 [1, 1376] = [hi block 688 | lo block 688]

Per-core pipeline (PE does nothing but matmuls):
  phase 1 (per 128-row i-block k): DMA q/s/z on SP queue; Act converts s,z
      to fp16; DVE extracts nibbles (1-byte shr/and), then fused
      (nib - z) * s in fp16 into resident WT[128, 32, 1376] fp16.
  phase 2 (per 128-token tile, x-prep prefetched 2 tiles ahead on Act
      queue): DMA x, fp32->fp16 on Act, xbar DMA-transpose to x.T tiles,
      PSUM-accumulate out = bias + sum_k xT[k].T @ WT[k] (bias preloaded
      via K=1 ones x bias matmul), DVE copy PSUM->SBUF, store on SP queue.
"""

import numpy as np
from contextlib import ExitStack

import concourse.bacc as bacc
import concourse.bass as bass
import concourse.mybir as mybir
import concourse.tile as tile
from concourse.bass_utils import run_bass_kernel_spmd

dt = mybir.dt
Alu = mybir.AluOpType

TOKENS, IN_F, OUT_F, GS = 4096, 4096, 11008, 64
G = OUT_F * IN_F // GS            # 704512 quantization groups
J = G // IN_F                     # 172 groups per (g_row, i) plane
NCORES = 8
RPC = 4                           # W_q rows per core (both nibbles)
O_HALF = RPC * J                  # 688 output cols per nibble block
O_C = 2 * O_HALF                  # 1376 output cols per core
NT = TOKENS // 128                # 32 token tiles
NK = IN_F // 128                  # 32 contraction blocks
O_SPLITS = ((0, 512), (512, 512), (1024, 352))   # psum o-tiles (1 bank each)
XC = 2048                         # x i-chunk (half a row-block)
NH = IN_F // XC                   # chunks per row-block
LOOKAHEAD = 3                     # x-prep prefetch distance (t-tiles)

_CACHE = {}


def _build():
    nc = bacc.Bacc("TRN2", target_bir_lowering=False, debug=False,
                   num_devices=NCORES)

    x_d = nc.dram_tensor("x", [TOKENS, IN_F], dt.float32, kind="ExternalInput")
    q_d = nc.dram_tensor("wqt", [IN_F, O_HALF], dt.uint8, kind="ExternalInput")
    s_d = nc.dram_tensor("st", [IN_F, J], dt.float32, kind="ExternalInput")
    z_d = nc.dram_tensor("zt", [IN_F, J], dt.float32, kind="ExternalInput")
    b_d = nc.dram_tensor("bias", [1, O_C], dt.float32, kind="ExternalInput")
    o_d = nc.dram_tensor("out", [TOKENS, O_C], dt.float32, kind="ExternalOutput")

    with ExitStack() as ctx:
        tc = ctx.enter_context(tile.TileContext(nc))
        const = ctx.enter_context(tc.tile_pool(name="const", bufs=1))
        p1 = ctx.enter_context(tc.tile_pool(name="p1", bufs=2))
        px = ctx.enter_context(tc.tile_pool(name="px", bufs=3))
        pxt = ctx.enter_context(tc.tile_pool(name="pxt", bufs=8))
        po = ctx.enter_context(tc.tile_pool(name="po", bufs=2))
        pacc = ctx.enter_context(
            tc.tile_pool(name="pacc", bufs=2, space=bass.MemorySpace.PSUM))

        biasf = const.tile([1, O_C], dt.float32)
        nc.scalar.dma_start(biasf[:], b_d[:])
        biash = const.tile([1, O_C], dt.float16)
        nc.scalar.copy(biash[:], biasf[:])
        ones = const.tile([1, 128], dt.float16)
        nc.vector.memset(ones[:], 1.0)

        # resident transposed dequantized weights: [i-partition, k-block, o]
        WT = const.tile([128, NK, O_C], dt.float16)

        def prefetch(t):
            """x row-block t: load (Act q), fp16 convert (Act), xbar
            transpose (Act q) into a [128(i%128), 16(i//128), 128(tok)]
            x.T tile per chunk."""
            xts = []
            for h in range(NH):
                xr = px.tile([128, XC], dt.float32, tag="xr")
                nc.sync.dma_start(
                    xr[:], x_d[t * 128:(t + 1) * 128, h * XC:(h + 1) * XC])
                xh = px.tile([128, XC], dt.float16, tag="xh")
                nc.scalar.copy(xh[:], xr[:])
                xT = pxt.tile([128, XC // 128, 128], dt.float16, tag="xT")
                nc.scalar.dma_start_transpose(xT[:], xh[:])
                xts.append(xT)
            return xts

        # ---- phase 1: dequant W.T into resident fp16 WT (DVE only) ----
        #   hi = q >> 4, lo = q & 15;  w = (nib - z) * s
        for k in range(NK):
            i0 = k * 128
            st = p1.tile([128, J], dt.float32, tag="s")
            nc.sync.dma_start(st[:], s_d[i0:i0 + 128, :])
            zt = p1.tile([128, J], dt.float32, tag="z")
            nc.sync.dma_start(zt[:], z_d[i0:i0 + 128, :])
            q = p1.tile([128, O_HALF], dt.uint8, tag="q")
            nc.sync.dma_start(q[:], q_d[i0:i0 + 128, :])
            sh = p1.tile([128, J], dt.float16, tag="sh")
            nc.scalar.copy(sh[:], st[:])
            zh = p1.tile([128, J], dt.float16, tag="zh")
            nc.scalar.copy(zh[:], zt[:])
            sb = sh[:, None, :].broadcast_to([128, RPC, J])
            zb = zh[:, None, :].broadcast_to([128, RPC, J])
            hi_u = p1.tile([128, RPC, J], dt.uint8, tag="hi")
            nc.vector.tensor_single_scalar(
                hi_u[:], q[:].rearrange("p (r j) -> p r j", j=J), 4,
                Alu.logical_shift_right)
            lo_u = p1.tile([128, RPC, J], dt.uint8, tag="lo")
            nc.vector.tensor_single_scalar(
                lo_u[:], q[:].rearrange("p (r j) -> p r j", j=J), 15,
                Alu.bitwise_and)
            dhi = p1.tile([128, RPC, J], dt.float16, tag="dhi")
            nc.vector.scalar_tensor_tensor(
                dhi[:], hi_u[:], 1.0, zb, Alu.mult, Alu.subtract)
            dlo = p1.tile([128, RPC, J], dt.float16, tag="dlo")
            nc.vector.scalar_tensor_tensor(
                dlo[:], lo_u[:], 1.0, zb, Alu.mult, Alu.subtract)
            nc.vector.tensor_mul(
                WT[:, k, 0:O_HALF].rearrange("p (r j) -> p r j", j=J),
                dhi[:], sb)
            nc.vector.tensor_mul(
                WT[:, k, O_HALF:O_C].rearrange("p (r j) -> p r j", j=J),
                dlo[:], sb)

        # ---- phase 2: matmul over prefetched x.T tiles, psum->out ----
        inflight = [prefetch(t) for t in range(LOOKAHEAD)]
        for t in range(NT):
            if t + LOOKAHEAD < NT:
                inflight.append(prefetch(t + LOOKAHEAD))
            xts = inflight.pop(0)
            acc = []
            for p, (ob, on) in enumerate(O_SPLITS):
                a = pacc.tile([128, on], dt.float32, tag=f"a{p}")
                nc.tensor.matmul(
                    a[:], ones[0:1, :], biash[0:1, ob:ob + on],
                    start=True, stop=False)
                acc.append(a)
            for h in range(NH):
                for di in range(XC // 128):
                    k = h * (XC // 128) + di
                    for p, (ob, on) in enumerate(O_SPLITS):
                        nc.tensor.matmul(
                            acc[p][:], xts[h][:, di, :], WT[:, k, ob:ob + on],
                            start=False, stop=(k == NK - 1))
            for p, (ob, on) in enumerate(O_SPLITS):
                ot = po.tile([128, on], dt.float32, tag=f"o{p}")
                nc.vector.tensor_copy(ot[:], acc[p][:])
                nc.sync.dma_start(
                    o_d[t * 128:(t + 1) * 128, ob:ob + on], ot[:])

    nc.compile()
    return nc


def get_nc():
    if "nc" not in _CACHE:
        _CACHE["nc"] = _build()
    return _CACHE["nc"]


def make_in_maps(x, W_q, scale, zero, bias):
    x = np.ascontiguousarray(x, dtype=np.float32)
    W_q3 = np.asarray(W_q).astype(np.uint8).reshape(GS // 2, J, IN_F)
    s_t = np.ascontiguousarray(
        np.asarray(scale, dtype=np.float32).reshape(J, IN_F).T)
    z_t = np.ascontiguousarray(
        np.asarray(zero, dtype=np.float32).reshape(J, IN_F).T)
    bias = np.asarray(bias, dtype=np.float32)
    in_maps = []
    for c in range(NCORES):
        wqt = np.ascontiguousarray(
            W_q3[RPC * c:RPC * (c + 1)].transpose(2, 0, 1).reshape(IN_F, O_HALF))
        b2 = np.concatenate([
            bias[O_HALF * c:O_HALF * (c + 1)],
            bias[OUT_F // 2 + O_HALF * c:OUT_F // 2 + O_HALF * (c + 1)],
        ]).reshape(1, O_C)
        in_maps.append({
            "x": x, "wqt": wqt, "st": s_t, "zt": z_t, "bias": b2,
        })
    return in_maps


def assemble_out(results):
    out = np.empty((TOKENS, OUT_F), dtype=np.float32)
    for c in range(NCORES):
        r = results[c]["out"]
        out[:, O_HALF * c:O_HALF * (c + 1)] = r[:, :O_HALF]
        out[:, OUT_F // 2 + O_HALF * c:OUT_F // 2 + O_HALF * (c + 1)] = \
            r[:, O_HALF:]
    return out


def kernel(x, W_q, scale, zero, bias):
    nc = get_nc()
    in_maps = make_in_maps(x, W_q, scale, zero, bias)
    res = run_bass_kernel_spmd(nc, in_maps, list(range(NCORES)))
    return assemble_out(res.results)
